# revision 58
# baseline (speedup 1.0000x reference)
"""Deformable attention Trainium2 kernel (8-core SPMD).

Sharding: core c -> batch b=c//4, output row block R0=16*(c%4) (16 rows x 64
cols = 1024 px). Each core computes its (b, rows) slice of the full output for
all heads (the MLP mixes channels, not pixels).

Wall-clock is dominated by host prep + host<->device transfer, so the host
ships only disjoint slices, all bf16: q/k/v row-block views, raw-layout
offsets, a 1/8 shard of the weights, one packed bias tensor (~32 MB total vs
147 MB for a halo-replicated fp32-ish scheme). On device, the k/v row blocks
are AllGathered within each 4-core batch group and the weight shards over all
8 cores; the bilinear base grid, selector matrices and clamp constants are
baked into the NEFF as Const tensors; R0 and the canvas window base
w0 = 8*(c%4 + (c%4)//2) are derived from partition_id (f32->i32 copies round
to nearest on HW, so floors use the is_gt fixup).

Per (g,t) image (24 per core): 24 k-ch (+8 pad) live in a 34x97 zero-bordered
canvas holding image rows [w0, w0+32) (gather cost scales with canvas cells;
|off| < 7 keeps all in-image samples inside the window). Canvas fills read the
gathered k/v at runtime (partition-id-dependent) DRAM offsets. 4 images stack
into a 128-partition quad; one GPSIMD ap_gather per (qd, chunk) fetches all 4
bilinear corners. Sample order within a 576-sample chunk is k-major
(col = k*64 + w) so raw offsets reorder on device via strided DMA. q.k channel
reduction and coefficient replication run on the PE via 0/1 selector matmuls;
bilinear weights / softmax / weighted-v reduction run on DVE/ACT in
[24 img, sample] planes. MLP uses exact erf-gelu. Output returns as bf16.
"""

import sys

sys.path.insert(0, "/opt/trn_rl_repo")

import contextlib

import numpy as np
import ml_dtypes

import concourse.bass as bass
import concourse.mybir as mybir
import concourse.tile as tile
from concourse import bacc
from concourse.bass_utils import run_bass_kernel_spmd

F32 = mybir.dt.float32
F32R = mybir.dt.float32r
F16 = mybir.dt.float16
BF = mybir.dt.bfloat16
I16 = mybir.dt.int16
I32 = mybir.dt.int32
AL = mybir.AluOpType
ACTF = mybir.ActivationFunctionType
AX = mybir.AxisListType

B, C, H, W = 2, 288, 64, 64
T, G, K = 2, 12, 9
HD = C // G  # 24
RB, PX = 16, 16 * 64  # rows / pixels per core
NS = PX * K  # samples per image (px-major: (px, tap))
CR, CC = 34, 97  # rows 1..32 = image rows [w0, w0+32), rows 0/33 = zero pad
CN = CR * CC  # canvas cells (4753)
SCALE = float(HD) ** -0.5
NCH = 16  # sample chunks per image
CH = NS // NCH  # 576 samples
CHPX = PX // NCH  # 64 px
NW = CH // 16  # wrapped idx cols per chunk

_WTOT = 3 * C * C + 2 * C * 2 * C  # 580608 weight elements
_WSH = _WTOT // 8

_CACHE = {}
DBG = False


def _const_arrays():
    """Static data baked into the NEFF as Const tensors."""
    c = {}
    # bilinear base grid, f16 (integers <= 145, exact): rows 0-63 y-part
    # (py + ky + 63, R0 added at runtime), rows 64-127 x-part (px + kx + 63).
    # sample order: col = py*576 + k*64 + w (k-major within each py chunk)
    ky = np.repeat(np.arange(3), 3).astype(np.float32)
    kx = np.tile(np.arange(3), 3).astype(np.float32)
    py = np.arange(RB, dtype=np.float32)
    pxc = np.arange(W, dtype=np.float32)
    base = np.zeros((128, NS), ml_dtypes.bfloat16)
    yv = py[:, None, None] + ky[None, :, None] + 63.0  # (16,9,1)
    base[0:64, :] = np.broadcast_to(yv, (RB, K, W)).reshape(NS)[None, :]
    xv = pxc[None, None, :] + kx[None, :, None] + 63.0  # (1,9,64)
    base[64:128, :] = np.broadcast_to(xv, (RB, K, W)).reshape(NS)[None, :]
    c["base16"] = base
    sel4 = np.zeros((128, 4), ml_dtypes.bfloat16)
    for j in range(4):
        sel4[32 * j : 32 * j + 24, j] = 1.0
    c["sel4"] = sel4
    selrep = np.zeros((64, 3 * 128), np.float16)
    for ti in range(2):
        for qd3 in range(3):
            for p in range(128):
                selrep[32 * ti + 4 * qd3 + p // 32, qd3 * 128 + p] = 1.0
    c["selrep"] = selrep
    selv4 = np.zeros((128, 4 * HD), ml_dtypes.bfloat16)
    for j in range(4):
        for dd in range(HD):
            selv4[32 * j + dd, HD * j + dd] = 1.0
    c["selv4"] = selv4
    chi = np.full((128, 1), 95.0, np.float32)
    chi[:64] = 32.0
    c["chi"] = chi
    csub = np.full((128, 1), 48.0, np.float32)
    csub[:64] = 63.0  # + runtime w0: canvas row = floor(y) - w0 + 1
    c["csub"] = csub
    ymask = np.zeros((128, 1), np.float32)
    ymask[:64] = 1.0
    c["ymask"] = ymask
    c["ones128"] = np.ones((1, 128), np.float32)
    return c


def build_program():
    nc = bacc.Bacc("TRN2", target_bir_lowering=False, debug=False, num_devices=8)

    def din(name, shape, dt=F32):
        return nc.dram_tensor(name, list(shape), dt, kind="ExternalInput").ap()

    io = {}
    io["q_in"] = din("q_in", (C, RB, W), BF)
    io["k_in"] = din("k_in", (T, C, RB, W), BF)
    io["v_in"] = din("v_in", (T, C, RB, W), BF)
    io["off_in"] = din("off_in", (T * G * K * 2, RB, W), BF)  # raw rows (t,g,k,a)
    # all 5 projection/MLP weights, transposed+flattened+sharded over the 8
    # cores (AllGathered on device): [wqt, wkt, wvt, w1t, w2t] row-major
    io["w_in"] = din("w_in", (1, _WSH), BF)
    # bias pack: cols 0-2 bq*SCALE, 3-8 bkvq (k cols 3-5, v cols 6-8),
    # 9-13 b1, 14-16 b2
    io["bias"] = din("bias", (128, 17))
    for name, arr in _const_arrays().items():
        io[name] = nc.inline_tensor(arr, name).ap()
    io["L4_d"] = nc.dram_tensor("L4_d", [64, 4 * NS], F16).ap()
    io["out_d"] = nc.dram_tensor("out", [C, PX], BF, kind="ExternalOutput").ap()
    io["pid"] = nc.partition_id_tensor
    if DBG:
        io["dbg_qp"] = nc.dram_tensor("dbg_qp", [128, 3 * PX], F32, kind="ExternalOutput").ap()
        io["dbg_oatt"] = nc.dram_tensor("dbg_oatt", [128, 3 * PX], F32, kind="ExternalOutput").ap()
        io["dbg_es"] = nc.dram_tensor("dbg_es", [64, NS], F32, kind="ExternalOutput").ap()
        io["dbg_pos"] = nc.dram_tensor("dbg_pos", [128, NS // 8], F32, kind="ExternalOutput").ap()

    with tile.TileContext(nc) as tc:
        _body(tc, nc, io)
    nc.compile()
    return nc


def _dma_to_chrows(sync, dst_tile, px, src_ap, ch0):
    """DMA src [24, px] into channel rows ch0..ch0+24 of a [128, 3*px] layout
    tile (ch c -> (c%128, c//128)), splitting at 128 boundaries."""
    lo, hi = ch0, ch0 + 24
    while lo < hi:
        kk = lo // 128
        r0 = lo - 128 * kk
        n = min(hi - lo, 128 - r0)
        s0 = lo - ch0
        sync.dma_start(
            out=dst_tile[r0 : r0 + n, kk * px : (kk + 1) * px],
            in_=src_ap[s0 : s0 + n, :],
        )
        lo += n


def _body(tc, nc, io):
    dve, act, gps, pe, sync = nc.vector, nc.scalar, nc.gpsimd, nc.tensor, nc.sync
    es = contextlib.ExitStack()
    ect = es.enter_context
    ctx = ect(contextlib.ExitStack())

    def mm(out, lhsT, rhs, start, stop):
        n = out.shape[-1]
        assert rhs.shape[-1] == n
        for c0 in range(0, n, 512):
            c1 = min(c0 + 512, n)
            pe.matmul(
                out[..., c0:c1], lhsT, rhs[..., c0:c1], start=start, stop=stop
            )

    def btap(ap2d, n, k):  # [p, n] -> [p, n, k] broadcast view
        return ap2d.unsqueeze(-1).to_broadcast([ap2d.shape[0], n, k])

    qf = io["q_in"].rearrange("c r w -> c (r w)")

    sb = ect(tc.tile_pool(name="persist", bufs=1))

    # ---------------- k/v row-block AllGather (halo exchange) ----------------
    # Each core ships only its own 16 rows; the 4 cores of a batch AllGather
    # them into the full (T, C, H, W) image in DRAM (rank-major row blocks).
    SKV = T * C * RB * W
    dram = ect(tc.tile_pool(name="dramp", bufs=1, space="DRAM"))
    kin_b = dram.tile([T, C * RB * W], BF, name="kin_b")
    vin_b = dram.tile([T, C * RB * W], BF, name="vin_b")
    kgt = nc.dram_tensor("kg", [4, SKV], BF, kind="Internal")
    vgt = nc.dram_tensor("vg", [4, SKV], BF, kind="Internal")
    kg, vg = kgt.ap(), vgt.ap()
    # weights: 1/8 shard per core, AllGather over all 8 -> full blob.
    # Issued first: the weight gather gates q/canvas projections, while the
    # k/v gathers only gate the K/V phases.
    win_b = dram.tile([1, _WSH], BF, name="win_b")
    wg = nc.dram_tensor("wg", [8, _WSH], BF, kind="Internal", addr_space="Shared").ap()
    sync.dma_start(out=win_b[:], in_=io["w_in"][:])
    gps.collective_compute(
        "AllGather", AL.bypass, replica_groups=[list(range(8))],
        ins=[win_b[:]], outs=[wg],
    )
    sync.dma_start(out=kin_b[:], in_=io["k_in"].rearrange("t c r w -> t (c r w)"))
    sync.dma_start(out=vin_b[:], in_=io["v_in"].rearrange("t c r w -> t (c r w)"))
    groups = [[0, 1, 2, 3], [4, 5, 6, 7]]
    gps.collective_compute(
        "AllGather", AL.bypass, replica_groups=groups, ins=[kin_b[:]], outs=[kg]
    )
    gps.collective_compute(
        "AllGather", AL.bypass, replica_groups=groups, ins=[vin_b[:]], outs=[vg]
    )
    # canvas reads image rows [w0, w0+32), w0 = 8*(c%4 + (c%4)//2): per-chunk
    # runtime element offsets into the rank-major (4, T, C, 16, W) gather
    pid_sc = sync.partition_id()
    m_sc = pid_sc % 4
    w0_sc = (m_sc + m_sc // 2) * 8
    CRW = C * RB * W
    coffs = []
    for ch in range(4):
        yk = w0_sc + 8 * ch
        coffs.append((yk // 16) * SKV + (yk % 16) * W)

    wflat = wg.rearrange("g n -> (g n)")
    o0, o1, o2, o3 = C * C, 2 * C * C, 3 * C * C, 3 * C * C + 2 * C * C
    io["wqt"] = wflat[0:o0].rearrange("(r c) -> r c", r=C)
    io["wkt"] = wflat[o0:o1].rearrange("(r c) -> r c", r=C)
    io["wvt"] = wflat[o1:o2].rearrange("(r c) -> r c", r=C)
    io["w1t"] = wflat[o2:o3].rearrange("(r c) -> r c", r=C)
    io["w2t"] = wflat[o3:_WTOT].rearrange("(r c) -> r c", r=2 * C)

    # ---------------- per-core scalars from partition id ----------------
    # R0m: rows<64 = 16*(c%4), else 0.
    ymask_s = sb.tile([128, 1], F32, name="ymask_s")
    sync.dma_start(out=ymask_s[:], in_=io["ymask"][:])
    chi_s = sb.tile([128, 1], F32, name="chi_s")
    sync.dma_start(out=chi_s[:], in_=io["chi"][:])
    csub_s = sb.tile([128, 1], F32, name="csub_s")
    sync.dma_start(out=csub_s[:], in_=io["csub"][:])
    R0m = sb.tile([128, 1], F32, name="R0m")
    with (
        tc.tile_pool(name="pidp", bufs=1) as pp,
        tc.tile_pool(name="pidps", bufs=1, space="PSUM") as ppp,
    ):
        ones_s = pp.tile([1, 128], F32, name="ones_s")
        sync.dma_start(out=ones_s[:], in_=io["ones128"][:])
        pid_i = pp.tile([1, 1], mybir.dt.uint32, name="pid_i")
        sync.dma_start(out=pid_i[:], in_=io["pid"][0:1, 0:1])
        pid_f = pp.tile([1, 1], F32, name="pid_f")
        dve.tensor_copy(out=pid_f[:], in_=pid_i[:])
        pidb_ps = ppp.tile([128, 1], F32, name="pidb_ps")
        pe.matmul(pidb_ps[:, :], ones_s[:, :], pid_f[:, :], start=True, stop=True)
        pidb = pp.tile([128, 1], F32, name="pidb")
        act.copy(pidb[:], pidb_ps[:, :])
        t1 = pp.tile([128, 1], F32, name="t1")
        t0 = pp.tile([128, 1], F32, name="t0")
        tg_ = pp.tile([128, 1], F32, name="tg_")
        t1i = pp.tile([128, 1], I32, name="t1i")
        c4 = pp.tile([128, 1], F32, name="c4")

        def floor_fix(dst, src):  # dst = floor(src); HW f32->i32 rounds-to-nearest
            dve.tensor_copy(out=t1i[:], in_=src)
            dve.tensor_copy(out=dst, in_=t1i[:])
            dve.tensor_tensor(out=tg_[:], in0=dst, in1=src, op=AL.is_gt)
            dve.tensor_tensor(out=dst, in0=dst, in1=tg_[:], op=AL.subtract)

        dve.tensor_scalar(out=t0[:], in0=pidb[:], scalar1=0.25, scalar2=None, op0=AL.mult)
        floor_fix(t1[:], t0[:])
        dve.tensor_scalar(out=t1[:], in0=t1[:], scalar1=-4.0, scalar2=None, op0=AL.mult)
        dve.tensor_tensor(out=c4[:], in0=pidb[:], in1=t1[:], op=AL.add)  # c%4
        dve.tensor_scalar(out=t1[:], in0=c4[:], scalar1=16.0, scalar2=None, op0=AL.mult)
        dve.tensor_tensor(out=R0m[:], in0=t1[:], in1=ymask_s[:], op=AL.mult)
        # csub_y += w0 = 8*(c%4 + (c%4)//2): canvas holds image rows [w0, w0+32)
        dve.tensor_scalar(out=t0[:], in0=c4[:], scalar1=0.5, scalar2=None, op0=AL.mult)
        floor_fix(t1[:], t0[:])
        dve.tensor_tensor(out=t0[:], in0=c4[:], in1=t1[:], op=AL.add)
        dve.tensor_scalar(out=t0[:], in0=t0[:], scalar1=8.0, scalar2=None, op0=AL.mult)
        dve.tensor_tensor(out=t0[:], in0=t0[:], in1=ymask_s[:], op=AL.mult)
        dve.tensor_tensor(out=csub_s[:], in0=csub_s[:], in1=t0[:], op=AL.add)

    # ---------------- weight/selector staging ----------------
    wk_s = sb.tile([128, 3 * C], BF, name="wk_s")
    wv_s = sb.tile([128, 3 * C], BF, name="wv_s")
    bias_s = sb.tile([128, 17], F32, name="bias_s")
    sync.dma_start(out=bias_s[:], in_=io["bias"][:])
    sel4_s = sb.tile([128, 4], BF, name="sel4_s")
    sync.dma_start(out=sel4_s[:], in_=io["sel4"][:])
    selrep_s = sb.tile([64, 3 * 128], F16, name="selrep_s")
    sync.dma_start(out=selrep_s[:], in_=io["selrep"][:])
    selv4_s = sb.tile([128, 4 * HD], BF, name="selv4_s")
    sync.dma_start(out=selv4_s[:], in_=io["selv4"][:])
    wrp = sb.tile([128, 6 * (NS // 16)], I16, name="wrp")
    oatt = sb.tile([128, 3 * PX], F32, name="oatt")
    act.memzero(oatt[:])

    wes = contextlib.ExitStack()  # weights live: build .. coef4
    pw = wes.enter_context(tc.tile_pool(name="pw", bufs=1))
    p_wy0 = pw.tile([64, NS], F16, name="p_wy0")
    p_wy1 = pw.tile([64, NS], F16, name="p_wy1")
    p_wxi = pw.tile([64, 2 * NS], F16, name="p_wxi")  # (wx0,wx1) interleaved
    idx_dram = nc.dram_tensor("idx_dram", [64, NS], I16).ap()

    # ---------------- offsets -> bilinear weights + wrapped indices ----------
    # off_in raw rows r = t*216 + g*18 + k*2 + a; sample col = py*576 + k*64 + w
    offr = io["off_in"].rearrange(
        "(t g k a) py w -> t a g py k w", t=T, g=G, k=K
    )
    QN = NS // 8
    with tc.tile_pool(name="wb", bufs=1) as wb:
        for qq in range(8):
            cs = slice(qq * QN, (qq + 1) * QN)
            offp = wb.tile([128, QN], BF, name="offp", tag="offp")
            act.memzero(offp[:])
            for a in range(2):
                for t in range(2):
                    r0 = a * 64 + t * 32
                    for pyl in range(2):
                        sync.dma_start(
                            out=offp[r0 : r0 + 12, pyl * 576 : (pyl + 1) * 576].rearrange(
                                "p (k w) -> p k w", k=K
                            ),
                            in_=offr[t, a, :, 2 * qq + pyl],
                        )
            basep = wb.tile([128, QN], BF, name="basep", tag="basep")
            sync.dma_start(out=basep[:], in_=io["base16"][:, cs])
            pos = wb.tile([128, QN], F32, name="pos", tag="pos")
            ii = wb.tile([128, QN], I32, name="ii", tag="ii")
            flo = wb.tile([128, QN], F32, name="flo", tag="flo")
            ta = wb.tile([128, QN], F32, name="ta", tag="ta")
            tb = wb.tile([128, QN], F32, name="tb", tag="tb")
            tg = wb.tile([128, QN], F32, name="tg", tag="tg")
            dve.tensor_tensor(out=pos[:], in0=offp[:], in1=basep[:], op=AL.add)
            dve.tensor_tensor(
                out=pos[:], in0=pos[:], in1=R0m[:].to_broadcast([128, QN]), op=AL.add
            )
            dve.tensor_copy(out=ii[:], in_=pos[:])
            dve.tensor_copy(out=flo[:], in_=ii[:])
            dve.tensor_tensor(out=ta[:], in0=flo[:], in1=pos[:], op=AL.is_gt)
            dve.tensor_tensor(out=flo[:], in0=flo[:], in1=ta[:], op=AL.subtract)
            dve.tensor_tensor(out=ta[:], in0=pos[:], in1=flo[:], op=AL.subtract)
            dve.tensor_scalar(out=tb[:], in0=flo[:], scalar1=64.0, scalar2=None, op0=AL.is_ge)
            dve.tensor_scalar(out=tg[:], in0=flo[:], scalar1=127.0, scalar2=None, op0=AL.is_le)
            dve.tensor_tensor(out=tb[:], in0=tb[:], in1=tg[:], op=AL.mult)
            dve.tensor_tensor(out=tg[:], in0=ta[:], in1=tb[:], op=AL.mult)
            dve.tensor_tensor(out=tb[:], in0=tb[:], in1=tg[:], op=AL.subtract)  # w0
            dve.tensor_copy(out=p_wy0[:, cs], in_=tb[:64, :])
            xsh = wb.tile([64, QN], F32, name="xsh", tag="xsh")
            sync.dma_start(out=xsh[:], in_=tb[64:128, :])
            dve.tensor_copy(
                out=p_wxi[:, 2 * qq * QN : 2 * (qq + 1) * QN].rearrange(
                    "p (n two) -> p n two", two=2
                )[:, :, 0],
                in_=xsh[:],
            )
            dve.tensor_scalar(out=tb[:], in0=flo[:], scalar1=63.0, scalar2=None, op0=AL.is_ge)
            dve.tensor_tensor(out=tb[:], in0=tb[:], in1=ta[:], op=AL.mult)
            dve.tensor_scalar(out=ta[:], in0=flo[:], scalar1=126.0, scalar2=None, op0=AL.is_le)
            dve.tensor_tensor(out=tb[:], in0=tb[:], in1=ta[:], op=AL.mult)  # w1
            dve.tensor_copy(out=p_wy1[:, cs], in_=tb[:64, :])
            xsh2 = wb.tile([64, QN], F32, name="xsh2", tag="xsh2")
            sync.dma_start(out=xsh2[:], in_=tb[64:128, :])
            dve.tensor_copy(
                out=p_wxi[:, 2 * qq * QN : 2 * (qq + 1) * QN].rearrange(
                    "p (n two) -> p n two", two=2
                )[:, :, 1],
                in_=xsh2[:],
            )
            dve.tensor_tensor(
                out=flo[:], in0=flo[:], in1=csub_s[:].to_broadcast([128, QN]), op=AL.subtract
            )
            dve.tensor_scalar(out=flo[:], in0=flo[:], scalar1=0.0, scalar2=None, op0=AL.max)
            dve.tensor_tensor(
                out=flo[:], in0=flo[:], in1=chi_s[:].to_broadcast([128, QN]), op=AL.min
            )
            xsh3 = wb.tile([64, QN], F32, name="xsh3", tag="xsh3")
            sync.dma_start(out=xsh3[:], in_=flo[64:128, :])
            dve.tensor_scalar(
                out=ta[:64, :], in0=flo[:64, :], scalar1=float(CC), scalar2=None, op0=AL.mult
            )
            dve.tensor_tensor(out=ta[:64, :], in0=ta[:64, :], in1=xsh3[:], op=AL.add)
            i16 = wb.tile([64, QN], I16, name="i16", tag="i16")
            dve.tensor_copy(out=i16[:], in_=ta[:64, :])
            sync.dma_start(out=idx_dram[:, cs], in_=i16[:])
            if DBG and qq == 0:
                sync.dma_start(out=io["dbg_pos"][:], in_=pos[:])
    for qd in range(6):
        for j in range(4):
            img = 32 * (qd // 3) + 4 * (qd % 3) + j
            sap = idx_dram[img : img + 1, :].rearrange("o (c p) -> (o p) c", p=16)
            sync.dma_start(
                out=wrp[32 * j : 32 * j + 16, qd * (NS // 16) : (qd + 1) * (NS // 16)], in_=sap
            )
            sync.dma_start(
                out=wrp[32 * j + 16 : 32 * j + 32, qd * (NS // 16) : (qd + 1) * (NS // 16)],
                in_=sap,
            )

    # wk/wv staging issued after the offsets stage so its DMAs (gated on the
    # weight AllGather) don't block the offsets pipeline in the DMA queues
    for i in range(3):
        n = min(128, C - 128 * i)
        sync.dma_start(out=wk_s[:n, i * C : (i + 1) * C], in_=io["wkt"][128 * i : 128 * i + n, :])
        sync.dma_start(out=wv_s[:n, i * C : (i + 1) * C], in_=io["wvt"][128 * i : 128 * i + n, :])

    # ---------------- q projection (scaled, bias folded) ----------------
    qes = contextlib.ExitStack()
    qpool = qes.enter_context(tc.tile_pool(name="qrep_pool", bufs=1))
    qrep = []
    with tc.tile_pool(name="qph", bufs=2) as qsc, tc.tile_pool(
        name="qph_ps", bufs=2, space="PSUM"
    ) as qpp:
        wq_s = qsc.tile([128, 3 * C], BF, name="wq_s", tag="wq")
        qp_s = qsc.tile([128, 3 * PX], F32, name="qp_s", tag="qp")
        for i in range(3):
            n = min(128, C - 128 * i)
            sync.dma_start(out=wq_s[:n, i * C : (i + 1) * C], in_=io["wqt"][128 * i : 128 * i + n, :])
        for m in range(3):
            mn = min(128, C - 128 * m)
            for nch in range(PX // 512):
                ps = qpp.tile([128, 512], F32, name="qps", tag="qps")
                for kk in range(3):
                    kn = min(128, C - 128 * kk)
                    rhs = qsc.tile([128, 512], BF, name="qrhs", tag=f"qrhs{kk}")
                    sync.dma_start(
                        out=rhs[:kn, :],
                        in_=qf[128 * kk : 128 * kk + kn, nch * 512 : nch * 512 + 512],
                    )
                    mm(
                        ps[:mn, :],
                        wq_s[:kn, kk * C + 128 * m : kk * C + 128 * m + mn],
                        rhs[:kn, :],
                        start=(kk == 0),
                        stop=(kk == 2),
                    )
                act.activation(
                    qp_s[:mn, m * PX + nch * 512 : m * PX + nch * 512 + 512],
                    ps[:mn, :],
                    ACTF.Identity,
                    bias=bias_s[:mn, m : m + 1],
                    scale=SCALE,
                )
        if DBG:
            sync.dma_start(out=io["dbg_qp"][:], in_=qp_s[:])

        def qch(c0, n):  # list of (qp_s row-slice) covering ch c0..c0+n
            out = []
            lo = c0
            while lo < c0 + n:
                kk = lo // 128
                r0 = lo - 128 * kk
                cnt = min(c0 + n - lo, 128 - r0)
                out.append(qp_s[r0 : r0 + cnt, kk * PX : kk * PX + PX])
                lo += cnt
            return out

        for qd3 in range(3):
            qr = qpool.tile([128, PX], F32, name=f"qrep{qd3}")
            for j in range(4):
                g = 4 * qd3 + j
                r = 32 * j
                for piece in qch(24 * g, 24):
                    np_ = piece.shape[0]
                    sync.dma_start(out=qr[r : r + np_, :], in_=piece)
                    r += np_
                for piece in qch(24 * g, 8):
                    np_ = piece.shape[0]
                    sync.dma_start(out=qr[r : r + np_, :], in_=piece)
                    r += np_
            qrep.append(qr)

    # ---------------- canvas construction ----------------
    def make_canvas(cvp, scp, cpp, which, qd):
        wmat = wk_s if which == 0 else wv_s
        srct = kgt if which == 0 else vgt
        ti, qd3 = qd // 3, qd % 3
        canq = cvp.tile([128, CN], F32, name="canq", tag="canq")
        act.memzero(canq[:])
        for nch in range(4):
            ps = cpp.tile([96, 512], F32, name="cvps", tag="cvps")
            for kk in range(3):
                kn = min(128, C - 128 * kk)
                rhs = scp.tile([128, 512], BF, name="cvrhs", tag=f"cvrhs{kk}")
                sync.dma_start(
                    out=rhs[:kn, :],
                    in_=bass.AP(
                        srct,
                        coffs[nch] + ti * CRW + 128 * kk * (RB * W),
                        [[RB * W, kn], [1, 512]],
                    ),
                )
                mm(
                    ps[:, :],
                    wmat[:kn, kk * C + 96 * qd3 : kk * C + 96 * qd3 + 96],
                    rhs[:kn, :],
                    start=(kk == 0),
                    stop=(kk == 2),
                )
            stg = scp.tile([96, 512], F32, name="cvstg", tag="cvstg")
            act.activation(
                stg[:, :], ps[:, :], ACTF.Identity,
                bias=bias_s[:96, 3 + which * 3 + qd3 : 3 + which * 3 + qd3 + 1], scale=1.0
            )
            for j in range(4):
                dst = canq[32 * j : 32 * j + 24, :].rearrange("p (r c) -> p r c", r=CR)[
                    :, 1 + nch * 8 : 1 + nch * 8 + 8, 16:80
                ]
                sync.dma_start(
                    out=dst,
                    in_=stg[24 * j : 24 * j + 24, :].rearrange("p (r c) -> p r c", r=8),
                )
        return canq

    # ---------------- K phase ----------------
    with (
        tc.tile_pool(name="kcv", bufs=2) as kcv,
        tc.tile_pool(name="ksc", bufs=2) as ksc,
        tc.tile_pool(name="kpp", bufs=2, space="PSUM") as kpp,
    ):
        for qd in range(6):
            qd3 = qd % 3
            canq = make_canvas(kcv, ksc, kpp, 0, qd)
            for chunk in range(NCH):
                wsl = wrp[:, qd * (NS // 16) + chunk * NW : qd * (NS // 16) + (chunk + 1) * NW]
                l4t = ksc.tile([4, 4 * CH], F16, name="l4t", tag="l4t")
                l4v = l4t[:].rearrange("p (n four) -> p four n", four=4)
                it4 = ksc.tile([128, 4 * NW], I16, name="it4", tag="it4")
                for ci, dlt in enumerate((0, 1, CC, CC + 1)):
                    dve.tensor_scalar(
                        out=it4[:, ci * NW : (ci + 1) * NW], in0=wsl,
                        scalar1=dlt, scalar2=None, op0=AL.add,
                    )
                gt4 = ksc.tile([128, 4 * CH], F32, name="gt4", tag="gt4")
                gps.ap_gather(gt4[:], canq[:].unsqueeze(-1), it4[:], 128, CN, 1, 4 * CH)
                gtb4 = ksc.tile([128, 4 * CH], BF, name="gtb4", tag="gtb4")
                dve.tensor_tensor(
                    out=gtb4[:].rearrange("p (c k n) -> p c k n", c=4, k=K),
                    in0=gt4[:].rearrange("p (c k n) -> p c k n", c=4, k=K),
                    in1=qrep[qd3][:, chunk * CHPX : (chunk + 1) * CHPX]
                    .unsqueeze(1)
                    .unsqueeze(1)
                    .to_broadcast([128, 4, K, CHPX]),
                    op=AL.mult,
                )
                for ci in range(4):
                    lps = kpp.tile([4, CH], F32, name="lps", tag="lps")
                    mm(
                        lps[:, :], sel4_s[:, :],
                        gtb4[:, ci * CH : (ci + 1) * CH], start=True, stop=True,
                    )
                    act.copy(l4v[:, ci, :], lps[:, :])
                im0 = 32 * (qd // 3) + 4 * (qd % 3)
                sync.dma_start(
                    out=io["L4_d"][im0 : im0 + 4, 4 * chunk * CH : 4 * (chunk + 1) * CH],
                    in_=l4t[:],
                )

    qes.close()

    # ---------------- lerp corner logits + softmax + coef4 ----------------
    ces = contextlib.ExitStack()  # e_s lives: lerp .. coef4
    pe_pool = ces.enter_context(tc.tile_pool(name="pe_s", bufs=1))
    e_s = pe_pool.tile([64, NS], F32, name="e_s")
    with tc.tile_pool(name="lrp", bufs=1) as lrp:
        for qq in range(8):
            cs = slice(qq * QN, (qq + 1) * QN)
            l4 = lrp.tile([64, 4 * QN], F16, name="l4", tag="l4")
            act.memzero(l4[:])
            sync.dma_start(out=l4[0:12, :], in_=io["L4_d"][0:12, 4 * qq * QN : 4 * (qq + 1) * QN])
            sync.dma_start(out=l4[32:44, :], in_=io["L4_d"][32:44, 4 * qq * QN : 4 * (qq + 1) * QN])
            l4q = l4[:].rearrange("p (n four) -> p n four", four=4)
            ybl = lrp.tile([64, 2 * QN], F32, name="ybl", tag="ybl")
            tmp = lrp.tile([64, 2 * QN], F32, name="tmp", tag="tmp")
            dve.tensor_tensor(
                out=ybl[:].rearrange("p (n two) -> p n two", two=2),
                in0=l4q[:, :, 0:2],
                in1=btap(p_wy0[:, cs], QN, 2),
                op=AL.mult,
            )
            dve.tensor_tensor(
                out=tmp[:].rearrange("p (n two) -> p n two", two=2),
                in0=l4q[:, :, 2:4],
                in1=btap(p_wy1[:, cs], QN, 2),
                op=AL.mult,
            )
            dve.tensor_tensor(out=ybl[:], in0=ybl[:], in1=tmp[:], op=AL.add)
            dve.tensor_tensor(
                out=ybl[:],
                in0=ybl[:],
                in1=p_wxi[:, 2 * qq * QN : 2 * (qq + 1) * QN],
                op=AL.mult,
            )
            dve.tensor_reduce(
                out=e_s[:, cs],
                in_=ybl[:].rearrange("p (n two) -> p n two", two=2),
                axis=AX.X,
                op=AL.add,
            )
    def esv(ap):  # e_s-layout view [p, chunk, w, k] (k strided, reduce-ready)
        return ap.rearrange("p (a k c) -> p a c k", a=NCH, k=K)

    def pxb(ap2d):  # [p, PX] -> broadcast over k: [p, chunk, w, k]
        return (
            ap2d.rearrange("p (a c) -> p a c", a=NCH)
            .unsqueeze(-1)
            .to_broadcast([ap2d.shape[0], NCH, W, K])
        )

    with tc.tile_pool(name="smx", bufs=1) as smx:
        m9 = smx.tile([64, PX], F32, name="m9")
        dve.tensor_reduce(out=m9[:], in_=esv(e_s[:]), axis=AX.X, op=AL.max)
        msx = smx.tile([64, PX], F32, name="msx")
        act.memzero(msx[:])
        mt = smx.tile([12, PX], F32, name="mt")
        sync.dma_start(out=mt[:], in_=m9[32:44, :])
        dve.tensor_tensor(out=msx[0:12, :], in0=m9[0:12, :], in1=mt[:], op=AL.max)
        sync.dma_start(out=msx[32:44, :], in_=msx[0:12, :])
        dve.tensor_tensor(
            out=esv(e_s[:]), in0=esv(e_s[:]), in1=pxb(msx[:]), op=AL.subtract
        )
        act.activation(e_s[:], e_s[:], ACTF.Exp)
        s9 = smx.tile([64, PX], F32, name="s9")
        dve.tensor_reduce(out=s9[:], in_=esv(e_s[:]), axis=AX.X, op=AL.add)
        ssx = smx.tile([64, PX], F32, name="ssx")
        act.memzero(ssx[:])
        st = smx.tile([12, PX], F32, name="st")
        sync.dma_start(out=st[:], in_=s9[32:44, :])
        dve.tensor_tensor(out=ssx[0:12, :], in0=s9[0:12, :], in1=st[:], op=AL.add)
        dve.reciprocal(out=ssx[0:12, :], in_=ssx[0:12, :])
        sync.dma_start(out=ssx[32:44, :], in_=ssx[0:12, :])
        dve.tensor_tensor(
            out=esv(e_s[:]), in0=esv(e_s[:]), in1=pxb(ssx[:]), op=AL.mult
        )
    if DBG:
        sync.dma_start(out=io["dbg_es"][:], in_=e_s[:])

    coef4_d = nc.dram_tensor("coef4_d", [64, 4 * NS], F16).ap()
    with tc.tile_pool(name="cfb", bufs=2) as cfb:
        for qq in range(8):
            cs = slice(qq * QN, (qq + 1) * QN)
            ca = cfb.tile([64, QN], F32, name="ca", tag="ca")
            cb = cfb.tile([64, QN], F32, name="cb", tag="cb")
            dve.tensor_tensor(out=ca[:], in0=e_s[:, cs], in1=p_wy0[:, cs], op=AL.mult)
            dve.tensor_tensor(out=cb[:], in0=e_s[:, cs], in1=p_wy1[:, cs], op=AL.mult)
            c4t = cfb.tile([64, 4 * QN], F16, name="c4t", tag="c4t")
            c4 = c4t[:].rearrange("p (n four) -> p n four", four=4)
            wxi = p_wxi[:, 2 * qq * QN : 2 * (qq + 1) * QN].rearrange(
                "p (n two) -> p n two", two=2
            )
            dve.tensor_tensor(out=c4[:, :, 0:2], in0=btap(ca[:], QN, 2), in1=wxi, op=AL.mult)
            dve.tensor_tensor(out=c4[:, :, 2:4], in0=btap(cb[:], QN, 2), in1=wxi, op=AL.mult)
            sync.dma_start(out=coef4_d[:, 4 * qq * QN : 4 * (qq + 1) * QN], in_=c4t[:])
    ces.close()
    wes.close()

    # ---------------- V phase ----------------
    with (
        tc.tile_pool(name="vcv", bufs=2) as vcv,
        tc.tile_pool(name="vsc", bufs=2) as vsc,
        tc.tile_pool(name="vpp", bufs=1, space="PSUM") as vpp,
        tc.tile_pool(name="vpp2", bufs=2, space="PSUM") as vpp2,
    ):
        for qd3 in range(3):
            otmp = [
                vsc.tile([24, PX], F32, name=f"otmp{j}", tag=f"otmp{j}") for j in range(4)
            ]
            for ti in range(T):
                qd = 3 * ti + qd3
                canq = make_canvas(vcv, vsc, vpp2, 1, qd)
                red = vsc.tile([128, PX], F32, name="red", tag="red")
                for chunk in range(NCH):
                    wsl = wrp[
                        :, qd * (NS // 16) + chunk * NW : qd * (NS // 16) + (chunk + 1) * NW
                    ]
                    # mall cols = (corner, k, w): (corner,k) merge to one
                    # stride-64 dim for the per-pixel reduce
                    mall = vsc.tile([128, 4 * CH], F32, name="mall", tag="mall")
                    cft = vsc.tile([64, 4 * CH], F16, name="cft", tag="cft")
                    sync.dma_start(
                        out=cft[:], in_=coef4_d[:, 4 * chunk * CH : 4 * (chunk + 1) * CH]
                    )
                    cfv = cft[:].rearrange("p (n four) -> p four n", four=4)
                    it4 = vsc.tile([128, 4 * NW], I16, name="vit4", tag="vit4")
                    for ci, dlt in enumerate((0, 1, CC, CC + 1)):
                        dve.tensor_scalar(
                            out=it4[:, ci * NW : (ci + 1) * NW], in0=wsl,
                            scalar1=dlt, scalar2=None, op0=AL.add,
                        )
                    gt4 = vsc.tile([128, 4 * CH], F32, name="vgt4", tag="vgt4")
                    gps.ap_gather(
                        gt4[:], canq[:].unsqueeze(-1), it4[:], 128, CN, 1, 4 * CH
                    )
                    for ci in range(4):
                        crp = vpp.tile([128, CH], F32, name="crp", tag="crp")
                        mm(
                            crp[:, :],
                            selrep_s[32 * ti : 32 * ti + 12, qd3 * 128 : qd3 * 128 + 128],
                            cfv[32 * ti : 32 * ti + 12, ci, :],
                            start=True,
                            stop=True,
                        )
                        dve.tensor_tensor(
                            out=mall[:, ci * CH : (ci + 1) * CH],
                            in0=gt4[:, ci * CH : (ci + 1) * CH],
                            in1=crp[:, :],
                            op=AL.mult,
                        )
                    dve.tensor_reduce(
                        out=red[:, chunk * CHPX : (chunk + 1) * CHPX],
                        in_=mall[:].rearrange("p (fk w) -> p w fk", w=W),
                        axis=AX.X,
                        op=AL.add,
                    )
                redb = vsc.tile([128, PX], BF, name="redb", tag="redb")
                act.copy(redb[:], red[:])
                for j in range(4):
                    vt = vpp2.tile([24, PX], F32, name="vt", tag="vt")
                    mm(
                        vt[:, :],
                        selv4_s[:, HD * j : HD * j + HD],
                        redb[:, :],
                        start=True,
                        stop=True,
                    )
                    if ti == 0:
                        dve.tensor_copy(out=otmp[j][:], in_=vt[:, :])
                    else:
                        dve.tensor_tensor(out=otmp[j][:], in0=otmp[j][:], in1=vt[:, :], op=AL.add)
            for j in range(4):
                g = 4 * qd3 + j
                _dma_to_chrows(sync, oatt, PX, otmp[j][:], 24 * g)

    if DBG:
        sync.dma_start(out=io["dbg_oatt"][:], in_=oatt[:])

    # ---------------- MLP (exact gelu) + residual ----------------
    with (
        tc.tile_pool(name="mlp", bufs=2) as mp,
        tc.tile_pool(name="mlps", bufs=1) as mps,
        tc.tile_pool(name="mpp", bufs=2, space="PSUM") as mpp,
    ):
        oattb = mps.tile([128, 3 * PX], BF, name="oattb")
        dve.tensor_copy(out=oattb[:], in_=oatt[:])
        w1_s = mps.tile([128, 3 * 2 * C], BF, name="w1_s")
        w2_s = mps.tile([128, 5 * C], BF, name="w2_s")
        h_s = mps.tile([128, 5 * PX], BF, name="h_s")
        for i in range(3):
            n = min(128, C - 128 * i)
            sync.dma_start(
                out=w1_s[:n, i * 2 * C : (i + 1) * 2 * C],
                in_=io["w1t"][128 * i : 128 * i + n, :],
            )
        for i in range(5):
            n = min(128, 2 * C - 128 * i)
            sync.dma_start(out=w2_s[:n, i * C : (i + 1) * C], in_=io["w2t"][128 * i : 128 * i + n, :])
        for m in range(5):
            mn = min(128, 2 * C - 128 * m)
            for nch in range(PX // 512):
                ps = mpp.tile([128, 512], F32, name="m1ps", tag="m1ps")
                for kk in range(3):
                    kn = min(128, C - 128 * kk)
                    mm(
                        ps[:mn, :],
                        w1_s[:kn, kk * 2 * C + 128 * m : kk * 2 * C + 128 * m + mn],
                        oattb[:kn, kk * PX + nch * 512 : kk * PX + nch * 512 + 512],
                        start=(kk == 0),
                        stop=(kk == 2),
                    )
                xg = mp.tile([128, 512], F32, name="xg", tag="xg")
                dve.tensor_tensor(
                    out=xg[:mn, :],
                    in0=ps[:mn, :],
                    in1=bias_s[:mn, 9 + m : 9 + m + 1].to_broadcast([mn, 512]),
                    op=AL.add,
                )
                er = mp.tile([128, 512], F32, name="er", tag="er")
                act.activation(
                    er[:mn, :], xg[:mn, :], ACTF.Erf, bias=0.0, scale=0.7071067811865476
                )
                dve.tensor_scalar(
                    out=er[:mn, :], in0=er[:mn, :], scalar1=1.0, scalar2=0.5, op0=AL.add, op1=AL.mult
                )
                dve.tensor_tensor(
                    out=h_s[:mn, m * PX + nch * 512 : m * PX + nch * 512 + 512],
                    in0=xg[:mn, :],
                    in1=er[:mn, :],
                    op=AL.mult,
                )
        for m in range(3):
            mn = min(128, C - 128 * m)
            for nch in range(PX // 512):
                ps = mpp.tile([128, 512], F32, name="m2ps", tag="m2ps")
                for kk in range(5):
                    kn = min(128, 2 * C - 128 * kk)
                    mm(
                        ps[:mn, :],
                        w2_s[:kn, kk * C + 128 * m : kk * C + 128 * m + mn],
                        h_s[:kn, kk * PX + nch * 512 : kk * PX + nch * 512 + 512],
                        start=(kk == 0),
                        stop=(kk == 4),
                    )
                og = mp.tile([128, 512], F32, name="og", tag="og")
                dve.tensor_tensor(
                    out=og[:mn, :],
                    in0=ps[:mn, :],
                    in1=bias_s[:mn, 14 + m : 14 + m + 1].to_broadcast([mn, 512]),
                    op=AL.add,
                )
                ogb = mp.tile([128, 512], BF, name="ogb", tag="ogb")
                dve.tensor_tensor(
                    out=ogb[:mn, :],
                    in0=og[:mn, :],
                    in1=oatt[:mn, m * PX + nch * 512 : m * PX + nch * 512 + 512],
                    op=AL.add,
                )
                sync.dma_start(
                    out=io["out_d"][128 * m : 128 * m + mn, nch * 512 : nch * 512 + 512],
                    in_=ogb[:mn, :],
                )
    es.close()


# ============================ host side ============================


def _host_inputs(q, k, v, offset, Wq, bq, Wk, bk, Wv, bv, W1, b1, W2, b2):
    BFnp = ml_dtypes.bfloat16
    qb = np.asarray(q).reshape(B, C, H, W).astype(BFnp)
    kb = np.asarray(k).astype(BFnp)  # (B,T,C,H,W)
    vb = np.asarray(v).astype(BFnp)
    # raw (t,g,k,axis)-row layout; reordering happens on device via DMA
    offt = np.asarray(offset).astype(BFnp)  # (B, 432, H, W)
    bias = np.zeros((128, 17), np.float32)
    bqs = np.asarray(bq) * SCALE
    b1a, b2a = np.asarray(b1), np.asarray(b2)
    bka, bva = np.asarray(bk), np.asarray(bv)
    for i in range(3):
        n = min(128, C - 128 * i)
        bias[:n, i] = bqs[128 * i : 128 * i + n]
        bias[:n, 14 + i] = b2a[128 * i : 128 * i + n]
    for qd3 in range(3):
        bias[:96, 3 + qd3] = bka[96 * qd3 : 96 * qd3 + 96]
        bias[:96, 6 + qd3] = bva[96 * qd3 : 96 * qd3 + 96]
    for i in range(5):
        n = min(128, 2 * C - 128 * i)
        bias[:n, 9 + i] = b1a[128 * i : 128 * i + n]
    wblob = np.concatenate(
        [
            np.asarray(Wq).T.astype(BFnp, order="C").reshape(-1),
            np.asarray(Wk).T.astype(BFnp, order="C").reshape(-1),
            np.asarray(Wv).T.astype(BFnp, order="C").reshape(-1),
            np.asarray(W1).T.astype(BFnp, order="C").reshape(-1),
            np.asarray(W2).T.astype(BFnp, order="C").reshape(-1),
        ]
    ).reshape(8, 1, _WSH)
    shared = {"bias": bias}
    cores = []
    for core in range(8):
        b, m = core // 4, core % 4
        R0 = 16 * m
        d = dict(shared)
        d["q_in"] = qb[b, :, R0 : R0 + RB]
        d["k_in"] = kb[b, :, :, R0 : R0 + RB]
        d["v_in"] = vb[b, :, :, R0 : R0 + RB]
        d["off_in"] = offt[b, :, R0 : R0 + RB]
        d["w_in"] = wblob[core]
        cores.append(d)
    return cores


def kernel(q, k, v, offset, Wq, bq, Wk, bk, Wv, bv, W1, b1, W2, b2):
    if "nc" not in _CACHE:
        _CACHE["nc"] = build_program()
    nc = _CACHE["nc"]
    ins = _host_inputs(q, k, v, offset, Wq, bq, Wk, bk, Wv, bv, W1, b1, W2, b2)
    res = run_bass_kernel_spmd(nc, ins, list(range(8))).results
    out = np.zeros((B, 1, C, H, W), np.float32)
    for core in range(8):
        b, R0 = core // 4, 16 * (core % 4)
        out[b, 0, :, R0 : R0 + RB, :] = res[core]["out"].reshape(C, RB, W).astype(np.float32)
    return out
